# revision 42
# baseline (speedup 1.0000x reference)
"""Trainium2 Bass kernel for the DigitCap forward pass.

Math note: in the reference, C = softmax(sum(A, axis=-2, keepdims=True), axis=-2)
is a softmax over a size-1 axis, so C == 1.0 exactly for any finite input.
The whole attention gram matrix cancels and the computation reduces to

    S[b,m,d] = sum_n (1 + B_prior[m,0,n]) * sum_p W[m,n,d,p] * u[b,n,p]
    out      = squash(S) = (1 - exp(-|S|)) * S / (|S| + 1e-7)

Sharding: M=10 digit caps are covered by 5 cores holding 2 caps each
(uniform SPMD program; the remaining 3 cores run duplicate pairs whose
outputs are discarded). No collectives needed.

Compute per core: contraction over (n,p)=9216 as 9 n-chunks. Each chunk
is ONE wide matmul: lhsT = uT chunk [n=128, (p,b)=128] (stationary),
rhs = W chunk [n=128, (p',m',d)=256] (moving), accumulating into
PSUM[(p,b)=128, (p',m',d)=256]. The p'==p diagonal blocks are the wanted
partial sums; they are extracted and summed over p in the epilogue
(8x streamed compute waste, but the PE is fed 256-wide which it likes).
"""

import os
import numpy as np

B = 16
N = 1152
DP = 8
M = 10
DD = 16
MS = 2           # m-slots per core
NCHUNK = N // 128
EPS = 1e-7

M_PAIRS = [(0, 1), (2, 3), (4, 5), (6, 7), (8, 9), (0, 1), (2, 3), (4, 5)]

_compiled = None


def _build_raw():
    """Raw (non-Tile) build: manual semaphores, no Tile entry/exit barriers.

    Pipeline: [sync] W(ms0)+out / [scalar] W(ms1)+uT+SEL+transcendentals /
    [gpsimd] cbt DMA only / [vector] cb1, uT f32r cast, 2/3 of scales,
    epilogue / [tensor] 9 wide f32r matmuls + 8 SEL-reduction matmuls.
    All DMA sources AND destinations are fully contiguous (host prepares
    group-major W layout; wt keeps ms outermost) so descriptors are multi-KB.
    """
    import concourse.bass as bass
    from concourse import bacc, mybir

    nc = bacc.Bacc("TRN2", target_bir_lowering=False, debug=False, num_devices=8)
    f32 = mybir.dt.float32
    f32r = mybir.dt.float32r
    AFT = mybir.ActivationFunctionType

    GROUPS = [(0, 3), (3, 6), (6, 9)]
    NG = len(GROUPS)

    # host layouts (see make_in_maps):
    #   W_h [ms, group-major flat (n', c_in_g, d*p)]
    #   u_h [n', c, p, b]   bp_h [n', c, m]
    w_d = nc.dram_tensor("W_h", [MS, N * DD * DP], f32, kind="ExternalInput")
    u_d = nc.dram_tensor("u_h", [128, NCHUNK, DP, B], f32, kind="ExternalInput")
    bp_d = nc.dram_tensor("bp_h", [128, NCHUNK, MS], f32, kind="ExternalInput")
    sel_d = nc.dram_tensor("SEL", [128, DP, B], f32, kind="ExternalInput")
    out_d = nc.dram_tensor("out_s", [B, MS, DD], f32, kind="ExternalOutput")
    out_ap = out_d.ap()

    from contextlib import ExitStack

    with ExitStack() as ctx:
        sb = lambda name, shape, dt_: ctx.enter_context(
            nc.sbuf_tensor(name, shape, dt_)
        )
        wt = sb("wt", [128, MS, NCHUNK, DD, DP], f32)
        wt_s = sb("wt_s", [128, MS, NCHUNK, DD, DP], f32r)
        ut = sb("ut", [128, NCHUNK, DP, B], f32)
        ut_r = sb("ut_r", [128, NCHUNK, DP, B], f32r)
        cbt = sb("cbt", [128, NCHUNK, MS], f32)
        cb1 = sb("cb1", [128, NCHUNK, MS], f32)
        sel = sb("sel", [128, DP, B], f32)
        ps_sb = sb("ps_sb", [128, DP, MS, DD], f32)
        s = sb("s", [B, MS, DD], f32)
        sq = sb("sq", [B, MS, DD], f32)
        n2 = sb("n2", [B, MS], f32)
        nrm = sb("nrm", [B, MS], f32)
        e_t = sb("e_t", [B, MS], f32)
        coef = sb("coef", [B, MS], f32)
        rec = sb("rec", [B, MS], f32)
        fac = sb("fac", [B, MS], f32)
        o = sb("o", [B, MS, DD], f32)
        warm = sb("warm", [B, MS], f32)
        ps = ctx.enter_context(nc.psum_tensor("ps", [128, DP, MS, DD], f32))
        ps2 = ctx.enter_context(nc.psum_tensor("ps2", [B, MS, DD], f32))
        sem = lambda name: ctx.enter_context(nc.semaphore(name))
        dcb, du, dsel, dos = sem("dcb"), sem("du"), sem("dsel"), sem("dos")
        dw = [[sem(f"dw{g}{m}") for m in range(MS)] for g in range(NG)]
        vs, ts, ss = sem("vs"), sem("ts"), sem("ss")
        scl = [sem(f"scl{c}") for c in range(NCHUNK)]

        # (c, ms) -> scale engine: 0=vector, 2=scalar (2:1 split; gpsimd
        # elementwise ops lock the shared DVE SBUF port - never use them)
        def eng_of(c, ms):
            return 2 if (2 * c + ms) % 3 == 2 else 0

        def grp_of(c):
            return next(g for g, (c0, c1) in enumerate(GROUPS) if c0 <= c < c1)

        with nc.Block() as block:

            def w_src(ms, g):
                c0, c1 = GROUPS[g]
                flat = w_d.ap()[ms, c0 * 128 * 128 : c1 * 128 * 128]
                return flat.rearrange("(n cdp) -> n cdp", n=128)

            def w_dst(ms, g):
                c0, c1 = GROUPS[g]
                return wt[:, ms, c0:c1].rearrange("n c d p -> n (c d p)")

            @block.sync
            def _(sync):
                for g in range(NG):
                    sync.dma_start(w_dst(0, g), w_src(0, g)).then_inc(dw[g][0], 16)
                sync.wait_ge(vs, 11)
                sync.dma_start(out_ap[:], o[:]).then_inc(dos, 16)
                sync.wait_ge(dos, 16)

            @block.gpsimd
            def _(gpsimd):
                gpsimd.dma_start(cbt[:], bp_d.ap()).then_inc(dcb, 16)

            @block.scalar
            def _(scalar):
                scalar.dma_start(w_dst(1, 0), w_src(1, 0)).then_inc(dw[0][1], 16)
                scalar.dma_start(
                    ut[:].rearrange("n c p b -> n c (p b)"),
                    u_d.ap().rearrange("n c p b -> n c (p b)"),
                ).then_inc(du, 16)
                scalar.dma_start(sel[:], sel_d.ap()).then_inc(dsel, 16)
                for g in range(1, NG):
                    scalar.dma_start(w_dst(1, g), w_src(1, g)).then_inc(
                        dw[g][1], 16
                    )
                # warm the Abs_reciprocal_sqrt table during the DMA phase
                # (input-independent: f(x*0+1)); Exp swaps once mid-epilogue.
                scalar.wait_ge(dcb, 16)
                nc.scalar.activation(
                    warm[:], cbt[0:B, 0], AFT.Abs_reciprocal_sqrt, bias=1.0,
                    scale=0.0,
                )
                scalar.wait_ge(vs, 1)
                for c in range(NCHUNK):
                    for ms in range(MS):
                        if eng_of(c, ms) != 2:
                            continue
                        scalar.wait_ge(dw[grp_of(c)][ms], 16)
                        nc.scalar.activation(
                            wt_s[:, ms, c],
                            wt[:, ms, c],
                            AFT.Copy,
                            scale=cb1[:, c, ms : ms + 1],
                        ).then_inc(scl[c])
                # epilogue transcendentals: r = 1/sqrt(n2), e = exp(-n2*r)
                scalar.wait_ge(vs, 6)
                nc.scalar.activation(rec[:], n2[:], AFT.Abs_reciprocal_sqrt).then_inc(
                    ss
                )
                scalar.wait_ge(vs, 7)
                nc.scalar.activation(e_t[:], nrm[:], AFT.Exp, scale=-1.0).then_inc(
                    ss
                )

            @block.vector
            def _(vector):
                vector.wait_ge(dcb, 16)
                nc.vector.tensor_scalar_add(cb1[:], cbt[:], 1.0).then_inc(vs)  # 1
                vector.wait_ge(vs, 1)  # cb1 is a PTR operand below
                # chunk-0 scales first so matmul 0 can fire asap
                for ms in range(MS):
                    vector.wait_ge(dw[0][ms], 16)
                    nc.vector.tensor_scalar_mul(
                        wt_s[:, ms, 0], wt[:, ms, 0], cb1[:, 0, ms : ms + 1]
                    ).then_inc(scl[0])
                vector.wait_ge(du, 16)
                nc.vector.tensor_copy(ut_r[:], ut[:]).then_inc(vs)  # 2 (f32r round)
                for c in range(1, NCHUNK):
                    for ms in range(MS):
                        if eng_of(c, ms) != 0:
                            continue
                        vector.wait_ge(dw[grp_of(c)][ms], 16)
                        nc.vector.tensor_scalar_mul(
                            wt_s[:, ms, c], wt[:, ms, c], cb1[:, c, ms : ms + 1]
                        ).then_inc(scl[c])
                vector.wait_ge(ts, 1)
                nc.vector.tensor_copy(ps_sb[:], ps[:]).then_inc(vs)  # 3
                vector.wait_ge(ts, 2)
                nc.vector.tensor_copy(s[:], ps2[:]).then_inc(vs)  # 4
                vector.wait_ge(vs, 4)
                nc.vector.tensor_mul(sq[:], s[:], s[:]).then_inc(vs)  # 5
                vector.wait_ge(vs, 5)
                nc.vector.tensor_reduce(
                    n2[:], sq[:], axis=mybir.AxisListType.X, op=mybir.AluOpType.add
                ).then_inc(vs)  # 6
                vector.wait_ge(ss, 1)
                nc.vector.tensor_mul(nrm[:], n2[:], rec[:]).then_inc(vs)  # 7
                vector.wait_ge(ss, 2)
                nc.vector.tensor_scalar(
                    coef[:], e_t[:], -1.0, 1.0, mybir.AluOpType.mult,
                    mybir.AluOpType.add,
                ).then_inc(vs)  # 8
                vector.wait_ge(vs, 8)
                nc.vector.tensor_mul(fac[:], coef[:], rec[:]).then_inc(vs)  # 9
                vector.wait_ge(vs, 9)  # fac is a PTR operand below
                nc.vector.tensor_scalar_mul(o[:, 0], s[:, 0], fac[:, 0:1]).then_inc(
                    vs
                )  # 10
                nc.vector.tensor_scalar_mul(o[:, 1], s[:, 1], fac[:, 1:2]).then_inc(
                    vs
                )  # 11

            @block.tensor
            def _(tensor):
                tensor.wait_ge(vs, 2)
                for c in range(NCHUNK):
                    tensor.wait_ge(scl[c], 2)
                    mm = nc.tensor.matmul(
                        ps[:],
                        ut_r[:, c].rearrange("n p b -> n (p b)"),
                        wt_s[:, :, c].rearrange("n m d p -> n p m d"),
                        start=(c == 0),
                        stop=(c == NCHUNK - 1),
                    )
                    if c == NCHUNK - 1:
                        mm.then_inc(ts)
                tensor.wait_ge(vs, 3)
                tensor.wait_ge(dsel, 16)
                for p in range(DP):
                    mm = nc.tensor.matmul(
                        ps2[:],
                        sel[:, p],
                        ps_sb[:, p],
                        start=(p == 0),
                        stop=(p == DP - 1),
                    )
                    if p == DP - 1:
                        mm.then_inc(ts)

    nc.compile()
    return nc


def _build():
    import concourse.bass as bass
    import concourse.tile as tile
    from concourse import bacc, mybir

    mm_dt = os.environ.get("KERNEL_MM_DT", "f32r")  # f32 | f32r | bf16
    n_wdma = int(os.environ.get("KERNEL_N_WDMA", "3"))  # W dma_start count

    nc = bacc.Bacc("TRN2", target_bir_lowering=False, debug=False, num_devices=8)
    f32 = mybir.dt.float32
    sb_dt = mybir.dt.bfloat16 if mm_dt == "bf16" else f32

    w_d = nc.dram_tensor("W_s", [MS, N, DD, DP], f32, kind="ExternalInput")
    u_d = nc.dram_tensor("uT", [N, DP, B], f32, kind="ExternalInput")
    bp_d = nc.dram_tensor("BpT", [N, MS], f32, kind="ExternalInput")
    sel_d = nc.dram_tensor("SEL", [128, DP, B], f32, kind="ExternalInput")
    out_d = nc.dram_tensor("out_s", [B, MS, DD], f32, kind="ExternalOutput")

    # source views, n-chunked to 128 partitions
    w_ap = w_d.ap().rearrange("m (c n) d p -> n c m (d p)", n=128)     # [128,9,2,128]
    u_ap = u_d.ap().rearrange("(c n) p b -> n c (p b)", n=128)         # [128,9,128]
    bp_ap = bp_d.ap().rearrange("(c n) m -> n c m", n=128)             # [128,9,2]
    out_ap = out_d.ap()

    with tile.TileContext(nc) as tc:
        with (
            tc.tile_pool(name="big", bufs=1) as big,
            tc.tile_pool(name="small", bufs=1) as small,
            tc.tile_pool(name="psum", bufs=1, space="PSUM") as psum,
        ):
            wt = big.tile([128, NCHUNK, MS, DD, DP], sb_dt, tag="wt")
            ut = big.tile([128, NCHUNK, DP, B], sb_dt, tag="ut")
            cbt = small.tile([128, NCHUNK, MS], f32, tag="cbt")
            sel = big.tile([128, DP, B], f32, tag="sel")
            dma_w = nc.gpsimd.dma_start if mm_dt == "bf16" else nc.sync.dma_start
            dma_u = nc.gpsimd.dma_start if mm_dt == "bf16" else nc.scalar.dma_start

            # tiny inputs first so cb1 and the first matmul aren't gated on
            # the big W transfers (HWDGE completion is FIFO per queue lane)
            nc.sync.dma_start(cbt[:], bp_ap)
            dma_u(ut[:].rearrange("n c p b -> n c (p b)"), u_ap)
            nc.scalar.dma_start(sel[:], sel_d.ap())

            # W: split into n_wdma issues so chunk-group g's matmuls can
            # start while group g+1 is still in flight
            assert NCHUNK % n_wdma == 0
            gsz = NCHUNK // n_wdma
            for g in range(n_wdma):
                for ms in range(MS):
                    if mm_dt == "bf16":
                        eng_dma = nc.gpsimd.dma_start
                    else:
                        eng_dma = nc.sync.dma_start if ms == 0 else nc.scalar.dma_start
                    eng_dma(
                        wt[:, g * gsz : (g + 1) * gsz, ms].rearrange(
                            "n c d p -> n c (d p)"
                        ),
                        w_ap[:, g * gsz : (g + 1) * gsz, ms],
                    )

            cb1 = small.tile([128, NCHUNK, MS], f32, tag="cb1")
            nc.vector.tensor_scalar_add(cb1[:], cbt[:], 1.0)

            # hoist ACT table loads (Sqrt/Exp, the only two ACT funcs used) so
            # they overlap the DMA phase instead of stalling the epilogue chain
            warm = small.tile([B, MS], f32, tag="warm")
            nc.scalar.activation(
                warm[:], cb1[0:B, 0], mybir.ActivationFunctionType.Exp, scale=-1.0
            )
            nc.scalar.activation(
                warm[:], warm[:], mybir.ActivationFunctionType.Sqrt
            )

            if mm_dt == "f32r":
                f32r = mybir.dt.float32r
                wt_s = big.tile([128, NCHUNK, MS, DD, DP], f32r, tag="wt_s")
                ut_mm = big.tile([128, NCHUNK, DP, B], f32r, tag="ut_mm")
                nc.vector.tensor_copy(ut_mm[:], ut[:])  # rounds to f32r
            else:
                wt_s = wt
                ut_mm = ut

            ps = psum.tile([128, DP, MS, DD], f32, tag="ps")
            for c in range(NCHUNK):
                # scale W by (1 + B_prior), per (n, chunk, m-slot); for f32r
                # this op also performs the required rounding on its output
                for ms in range(MS):
                    nc.vector.tensor_scalar_mul(
                        wt_s[:, c, ms], wt[:, c, ms], cb1[:, c, ms : ms + 1]
                    )
                nc.tensor.matmul(
                    ps[:],
                    ut_mm[:, c].rearrange("n p b -> n (p b)"),
                    wt_s[:, c].rearrange("n m d p -> n p m d"),
                    start=(c == 0),
                    stop=(c == NCHUNK - 1),
                )

            # diagonal extraction: S[b, m', d] = sum_p ps[16p+b, p, :, :].
            # DVE/walrus reject partition bases that aren't 32-aligned, so the
            # cross-partition gather runs on the PE: out2[b,:] accumulates
            # SEL[:, p].T @ ps_sb[:, p] over p, where SEL[q,p,b] = (q==16p+b).
            f32t = f32
            ps_sb = small.tile([128, DP, MS, DD], f32, tag="ps_sb")
            nc.vector.tensor_copy(ps_sb[:], ps[:])
            ps2 = psum.tile([B, MS, DD], f32, tag="ps2")
            for p in range(DP):
                nc.tensor.matmul(
                    ps2[:],
                    sel[:, p],
                    ps_sb[:, p],
                    start=(p == 0),
                    stop=(p == DP - 1),
                )
            s = small.tile([B, MS, DD], f32t, tag="s")
            nc.vector.tensor_copy(s[:], ps2[:])

            # squash over d per (b, m-slot)
            sq = small.tile([B, MS, DD], f32t, tag="sq")
            nc.vector.tensor_mul(sq[:], s[:], s[:])
            n2 = small.tile([B, MS], f32t, tag="n2")
            nc.vector.tensor_reduce(
                n2[:], sq[:], axis=mybir.AxisListType.X, op=mybir.AluOpType.add
            )
            nrm = small.tile([B, MS], f32t, tag="nrm")
            nc.scalar.sqrt(nrm[:], n2[:])
            e = small.tile([B, MS], f32t, tag="e")
            nc.scalar.activation(
                e[:], nrm[:], mybir.ActivationFunctionType.Exp, scale=-1.0
            )
            coef = small.tile([B, MS], f32t, tag="coef")
            nc.vector.tensor_scalar(
                coef[:], e[:], -1.0, 1.0, mybir.AluOpType.mult, mybir.AluOpType.add
            )
            neps = small.tile([B, MS], f32t, tag="neps")
            nc.vector.tensor_scalar_add(neps[:], nrm[:], EPS)
            rec = small.tile([B, MS], f32t, tag="rec")
            nc.vector.reciprocal(rec[:], neps[:])
            fac = small.tile([B, MS], f32t, tag="fac")
            nc.vector.tensor_mul(fac[:], coef[:], rec[:])
            o = small.tile([B, MS, DD], f32, tag="o")
            for ms in range(MS):
                nc.vector.tensor_scalar_mul(o[:, ms], s[:, ms], fac[:, ms : ms + 1])
            nc.sync.dma_start(out_ap[:], o[:])

    nc.compile()
    return nc


def make_in_maps(primary_caps: np.ndarray, W: np.ndarray, B_prior: np.ndarray):
    GROUPS = [(0, 3), (3, 6), (6, 9)]
    u = np.asarray(primary_caps, dtype=np.float32)
    # u_h [n', c, p, b]
    u_h = np.ascontiguousarray(
        u.transpose(1, 2, 0).reshape(NCHUNK, 128, DP, B).transpose(1, 0, 2, 3)
    )
    sel = np.zeros((128, DP, B), dtype=np.float32)
    for p in range(DP):
        for b in range(B):
            sel[16 * p + b, p, b] = 1.0
    Wf = np.asarray(W, dtype=np.float32)
    Bf = np.asarray(B_prior, dtype=np.float32)
    in_maps = []
    for pr in M_PAIRS:
        wp = Wf[list(pr)]  # [MS, N, DD, DP]
        # W_h [ms, flat group-major (n', c_in_g, d*p)]
        parts = []
        for ms in range(MS):
            row = []
            for c0, c1 in GROUPS:
                blk = wp[ms, c0 * 128 : c1 * 128].reshape(c1 - c0, 128, DD * DP)
                row.append(blk.transpose(1, 0, 2).reshape(-1))
            parts.append(np.concatenate(row))
        w_h = np.ascontiguousarray(np.stack(parts))
        bp = Bf[list(pr), 0, :]  # [MS, N]
        bp_h = np.ascontiguousarray(
            bp.T.reshape(NCHUNK, 128, MS).transpose(1, 0, 2)
        )
        in_maps.append({"W_h": w_h, "u_h": u_h, "bp_h": bp_h, "SEL": sel})
    return in_maps


def kernel(primary_caps: np.ndarray, W: np.ndarray, B_prior: np.ndarray) -> np.ndarray:
    from concourse.bass_utils import run_bass_kernel_spmd

    global _compiled
    if _compiled is None:
        _compiled = _build_raw()
    nc = _compiled

    in_maps = make_in_maps(primary_caps, W, B_prior)
    res = run_bass_kernel_spmd(nc, in_maps, list(range(8))).results
    out = np.empty((B, M, DD), dtype=np.float32)
    for i in range(5):
        out[:, 2 * i : 2 * i + 2, :] = res[i]["out_s"]
    return out


# revision 43
# speedup vs baseline: 1.0265x; 1.0265x over previous
"""Trainium2 Bass kernel for the DigitCap forward pass.

Math note: in the reference, C = softmax(sum(A, axis=-2, keepdims=True), axis=-2)
is a softmax over a size-1 axis, so C == 1.0 exactly for any finite input.
The whole attention gram matrix cancels and the computation reduces to

    S[b,m,d] = sum_n (1 + B_prior[m,0,n]) * sum_p W[m,n,d,p] * u[b,n,p]
    out      = squash(S) = (1 - exp(-|S|)) * S / (|S| + 1e-7)

Sharding: M=10 digit caps are covered by 5 cores holding 2 caps each
(uniform SPMD program; the remaining 3 cores run duplicate pairs whose
outputs are discarded). No collectives needed.

Compute per core: contraction over (n,p)=9216 as 9 n-chunks. Each chunk
is ONE wide matmul: lhsT = uT chunk [n=128, (p,b)=128] (stationary),
rhs = W chunk [n=128, (p',m',d)=256] (moving), accumulating into
PSUM[(p,b)=128, (p',m',d)=256]. The p'==p diagonal blocks are the wanted
partial sums; they are extracted and summed over p in the epilogue
(8x streamed compute waste, but the PE is fed 256-wide which it likes).
"""

import os
import numpy as np

B = 16
N = 1152
DP = 8
M = 10
DD = 16
MS = 2           # m-slots per core
NCHUNK = N // 128
EPS = 1e-7

M_PAIRS = [(0, 1), (2, 3), (4, 5), (6, 7), (8, 9), (0, 1), (2, 3), (4, 5)]

_compiled = None


def _build_raw():
    """Raw (non-Tile) build: manual semaphores, no Tile entry/exit barriers.

    Pipeline: [sync] W(ms0)+out / [scalar] W(ms1)+uT+SEL+transcendentals /
    [gpsimd] cbt DMA only / [vector] cb1, uT f32r cast, 2/3 of scales,
    epilogue / [tensor] 9 wide f32r matmuls + 8 SEL-reduction matmuls.
    All DMA sources AND destinations are fully contiguous (host prepares
    group-major W layout; wt keeps ms outermost) so descriptors are multi-KB.
    """
    import concourse.bass as bass
    from concourse import bacc, mybir

    nc = bacc.Bacc("TRN2", target_bir_lowering=False, debug=False, num_devices=8)
    f32 = mybir.dt.float32
    f32r = mybir.dt.float32r
    AFT = mybir.ActivationFunctionType

    GROUPS = [(0, 1), (1, 5), (5, 9)]
    NG = len(GROUPS)

    # host layouts (see make_in_maps):
    #   W_h [ms, group-major flat (n', c_in_g, d*p)]
    #   u_h [n', c, p, b]   bp_h [n', c, m]
    w_d = nc.dram_tensor("W_h", [MS, N * DD * DP], f32, kind="ExternalInput")
    u_d = nc.dram_tensor("u_h", [128, NCHUNK, DP, B], f32, kind="ExternalInput")
    bp_d = nc.dram_tensor("bp_h", [128, NCHUNK, MS], f32, kind="ExternalInput")
    sel_d = nc.dram_tensor("SEL", [128, DP, B], f32, kind="ExternalInput")
    out_d = nc.dram_tensor("out_s", [B, MS, DD], f32, kind="ExternalOutput")
    out_ap = out_d.ap()

    from contextlib import ExitStack

    with ExitStack() as ctx:
        sb = lambda name, shape, dt_: ctx.enter_context(
            nc.sbuf_tensor(name, shape, dt_)
        )
        wt = sb("wt", [128, MS, NCHUNK, DD, DP], f32)
        wt_s = sb("wt_s", [128, MS, NCHUNK, DD, DP], f32r)
        ut = sb("ut", [128, NCHUNK, DP, B], f32)
        ut_r = sb("ut_r", [128, NCHUNK, DP, B], f32r)
        cbt = sb("cbt", [128, NCHUNK, MS], f32)
        cb1 = sb("cb1", [128, NCHUNK, MS], f32)
        sel = sb("sel", [128, DP, B], f32)
        ps_sb = sb("ps_sb", [128, DP, MS, DD], f32)
        s = sb("s", [B, MS, DD], f32)
        sq = sb("sq", [B, MS, DD], f32)
        n2 = sb("n2", [B, MS], f32)
        nrm = sb("nrm", [B, MS], f32)
        e_t = sb("e_t", [B, MS], f32)
        coef = sb("coef", [B, MS], f32)
        rec = sb("rec", [B, MS], f32)
        fac = sb("fac", [B, MS], f32)
        o = sb("o", [B, MS, DD], f32)
        warm = sb("warm", [B, MS], f32)
        ps = ctx.enter_context(nc.psum_tensor("ps", [128, DP, MS, DD], f32))
        ps2 = ctx.enter_context(nc.psum_tensor("ps2", [B, MS, DD], f32))
        sem = lambda name: ctx.enter_context(nc.semaphore(name))
        dcb, du, dsel, dos = sem("dcb"), sem("du"), sem("dsel"), sem("dos")
        dw = [[sem(f"dw{g}{m}") for m in range(MS)] for g in range(NG)]
        vs, ts, ss = sem("vs"), sem("ts"), sem("ss")
        scl = [sem(f"scl{c}") for c in range(NCHUNK)]

        # (c, ms) -> scale engine: 0=vector, 2=scalar (2:1 split; gpsimd
        # elementwise ops lock the shared DVE SBUF port - never use them)
        def eng_of(c, ms):
            return 2 if (2 * c + ms) % 3 == 2 else 0

        def grp_of(c):
            return next(g for g, (c0, c1) in enumerate(GROUPS) if c0 <= c < c1)

        with nc.Block() as block:

            def w_src(ms, g):
                c0, c1 = GROUPS[g]
                flat = w_d.ap()[ms, c0 * 128 * 128 : c1 * 128 * 128]
                return flat.rearrange("(n cdp) -> n cdp", n=128)

            def w_dst(ms, g):
                c0, c1 = GROUPS[g]
                return wt[:, ms, c0:c1].rearrange("n c d p -> n (c d p)")

            @block.sync
            def _(sync):
                for g in range(NG):
                    sync.dma_start(w_dst(0, g), w_src(0, g)).then_inc(dw[g][0], 16)
                sync.wait_ge(vs, 11)
                sync.dma_start(out_ap[:], o[:]).then_inc(dos, 16)
                sync.wait_ge(dos, 16)

            @block.gpsimd
            def _(gpsimd):
                gpsimd.dma_start(cbt[:], bp_d.ap()).then_inc(dcb, 16)

            @block.scalar
            def _(scalar):
                scalar.dma_start(w_dst(1, 0), w_src(1, 0)).then_inc(dw[0][1], 16)
                scalar.dma_start(
                    ut[:].rearrange("n c p b -> n c (p b)"),
                    u_d.ap().rearrange("n c p b -> n c (p b)"),
                ).then_inc(du, 16)
                scalar.dma_start(sel[:], sel_d.ap()).then_inc(dsel, 16)
                for g in range(1, NG):
                    scalar.dma_start(w_dst(1, g), w_src(1, g)).then_inc(
                        dw[g][1], 16
                    )
                # warm the Abs_reciprocal_sqrt table during the DMA phase
                # (input-independent: f(x*0+1)); Exp swaps once mid-epilogue.
                scalar.wait_ge(dcb, 16)
                nc.scalar.activation(
                    warm[:], cbt[0:B, 0], AFT.Abs_reciprocal_sqrt, bias=1.0,
                    scale=0.0,
                )
                scalar.wait_ge(vs, 1)
                for c in range(NCHUNK):
                    for ms in range(MS):
                        if eng_of(c, ms) != 2:
                            continue
                        scalar.wait_ge(dw[grp_of(c)][ms], 16)
                        nc.scalar.activation(
                            wt_s[:, ms, c],
                            wt[:, ms, c],
                            AFT.Copy,
                            scale=cb1[:, c, ms : ms + 1],
                        ).then_inc(scl[c])
                # epilogue transcendentals: r = 1/sqrt(n2), e = exp(-n2*r)
                scalar.wait_ge(vs, 6)
                nc.scalar.activation(rec[:], n2[:], AFT.Abs_reciprocal_sqrt).then_inc(
                    ss
                )
                scalar.wait_ge(vs, 7)
                nc.scalar.activation(e_t[:], nrm[:], AFT.Exp, scale=-1.0).then_inc(
                    ss
                )

            @block.vector
            def _(vector):
                vector.wait_ge(dcb, 16)
                nc.vector.tensor_scalar_add(cb1[:], cbt[:], 1.0).then_inc(vs)  # 1
                vector.wait_ge(vs, 1)  # cb1 is a PTR operand below
                # chunk-0 scales first so matmul 0 can fire asap
                for ms in range(MS):
                    vector.wait_ge(dw[0][ms], 16)
                    nc.vector.tensor_scalar_mul(
                        wt_s[:, ms, 0], wt[:, ms, 0], cb1[:, 0, ms : ms + 1]
                    ).then_inc(scl[0])
                vector.wait_ge(du, 16)
                nc.vector.tensor_copy(ut_r[:], ut[:]).then_inc(vs)  # 2 (f32r round)
                for c in range(1, NCHUNK):
                    for ms in range(MS):
                        if eng_of(c, ms) != 0:
                            continue
                        vector.wait_ge(dw[grp_of(c)][ms], 16)
                        nc.vector.tensor_scalar_mul(
                            wt_s[:, ms, c], wt[:, ms, c], cb1[:, c, ms : ms + 1]
                        ).then_inc(scl[c])
                vector.wait_ge(ts, 1)
                nc.vector.tensor_copy(ps_sb[:], ps[:]).then_inc(vs)  # 3
                vector.wait_ge(ts, 2)
                nc.vector.tensor_copy(s[:], ps2[:]).then_inc(vs)  # 4
                vector.wait_ge(vs, 4)
                nc.vector.tensor_mul(sq[:], s[:], s[:]).then_inc(vs)  # 5
                vector.wait_ge(vs, 5)
                nc.vector.tensor_reduce(
                    n2[:], sq[:], axis=mybir.AxisListType.X, op=mybir.AluOpType.add
                ).then_inc(vs)  # 6
                vector.wait_ge(ss, 1)
                nc.vector.tensor_mul(nrm[:], n2[:], rec[:]).then_inc(vs)  # 7
                vector.wait_ge(ss, 2)
                nc.vector.tensor_scalar(
                    coef[:], e_t[:], -1.0, 1.0, mybir.AluOpType.mult,
                    mybir.AluOpType.add,
                ).then_inc(vs)  # 8
                vector.wait_ge(vs, 8)
                nc.vector.tensor_mul(fac[:], coef[:], rec[:]).then_inc(vs)  # 9
                vector.wait_ge(vs, 9)  # fac is a PTR operand below
                nc.vector.tensor_scalar_mul(o[:, 0], s[:, 0], fac[:, 0:1]).then_inc(
                    vs
                )  # 10
                nc.vector.tensor_scalar_mul(o[:, 1], s[:, 1], fac[:, 1:2]).then_inc(
                    vs
                )  # 11

            @block.tensor
            def _(tensor):
                tensor.wait_ge(vs, 2)
                for c in range(NCHUNK):
                    tensor.wait_ge(scl[c], 2)
                    mm = nc.tensor.matmul(
                        ps[:],
                        ut_r[:, c].rearrange("n p b -> n (p b)"),
                        wt_s[:, :, c].rearrange("n m d p -> n p m d"),
                        start=(c == 0),
                        stop=(c == NCHUNK - 1),
                    )
                    if c == NCHUNK - 1:
                        mm.then_inc(ts)
                tensor.wait_ge(vs, 3)
                tensor.wait_ge(dsel, 16)
                for p in range(DP):
                    mm = nc.tensor.matmul(
                        ps2[:],
                        sel[:, p],
                        ps_sb[:, p],
                        start=(p == 0),
                        stop=(p == DP - 1),
                    )
                    if p == DP - 1:
                        mm.then_inc(ts)

    nc.compile()
    return nc


def _build():
    import concourse.bass as bass
    import concourse.tile as tile
    from concourse import bacc, mybir

    mm_dt = os.environ.get("KERNEL_MM_DT", "f32r")  # f32 | f32r | bf16
    n_wdma = int(os.environ.get("KERNEL_N_WDMA", "3"))  # W dma_start count

    nc = bacc.Bacc("TRN2", target_bir_lowering=False, debug=False, num_devices=8)
    f32 = mybir.dt.float32
    sb_dt = mybir.dt.bfloat16 if mm_dt == "bf16" else f32

    w_d = nc.dram_tensor("W_s", [MS, N, DD, DP], f32, kind="ExternalInput")
    u_d = nc.dram_tensor("uT", [N, DP, B], f32, kind="ExternalInput")
    bp_d = nc.dram_tensor("BpT", [N, MS], f32, kind="ExternalInput")
    sel_d = nc.dram_tensor("SEL", [128, DP, B], f32, kind="ExternalInput")
    out_d = nc.dram_tensor("out_s", [B, MS, DD], f32, kind="ExternalOutput")

    # source views, n-chunked to 128 partitions
    w_ap = w_d.ap().rearrange("m (c n) d p -> n c m (d p)", n=128)     # [128,9,2,128]
    u_ap = u_d.ap().rearrange("(c n) p b -> n c (p b)", n=128)         # [128,9,128]
    bp_ap = bp_d.ap().rearrange("(c n) m -> n c m", n=128)             # [128,9,2]
    out_ap = out_d.ap()

    with tile.TileContext(nc) as tc:
        with (
            tc.tile_pool(name="big", bufs=1) as big,
            tc.tile_pool(name="small", bufs=1) as small,
            tc.tile_pool(name="psum", bufs=1, space="PSUM") as psum,
        ):
            wt = big.tile([128, NCHUNK, MS, DD, DP], sb_dt, tag="wt")
            ut = big.tile([128, NCHUNK, DP, B], sb_dt, tag="ut")
            cbt = small.tile([128, NCHUNK, MS], f32, tag="cbt")
            sel = big.tile([128, DP, B], f32, tag="sel")
            dma_w = nc.gpsimd.dma_start if mm_dt == "bf16" else nc.sync.dma_start
            dma_u = nc.gpsimd.dma_start if mm_dt == "bf16" else nc.scalar.dma_start

            # tiny inputs first so cb1 and the first matmul aren't gated on
            # the big W transfers (HWDGE completion is FIFO per queue lane)
            nc.sync.dma_start(cbt[:], bp_ap)
            dma_u(ut[:].rearrange("n c p b -> n c (p b)"), u_ap)
            nc.scalar.dma_start(sel[:], sel_d.ap())

            # W: split into n_wdma issues so chunk-group g's matmuls can
            # start while group g+1 is still in flight
            assert NCHUNK % n_wdma == 0
            gsz = NCHUNK // n_wdma
            for g in range(n_wdma):
                for ms in range(MS):
                    if mm_dt == "bf16":
                        eng_dma = nc.gpsimd.dma_start
                    else:
                        eng_dma = nc.sync.dma_start if ms == 0 else nc.scalar.dma_start
                    eng_dma(
                        wt[:, g * gsz : (g + 1) * gsz, ms].rearrange(
                            "n c d p -> n c (d p)"
                        ),
                        w_ap[:, g * gsz : (g + 1) * gsz, ms],
                    )

            cb1 = small.tile([128, NCHUNK, MS], f32, tag="cb1")
            nc.vector.tensor_scalar_add(cb1[:], cbt[:], 1.0)

            # hoist ACT table loads (Sqrt/Exp, the only two ACT funcs used) so
            # they overlap the DMA phase instead of stalling the epilogue chain
            warm = small.tile([B, MS], f32, tag="warm")
            nc.scalar.activation(
                warm[:], cb1[0:B, 0], mybir.ActivationFunctionType.Exp, scale=-1.0
            )
            nc.scalar.activation(
                warm[:], warm[:], mybir.ActivationFunctionType.Sqrt
            )

            if mm_dt == "f32r":
                f32r = mybir.dt.float32r
                wt_s = big.tile([128, NCHUNK, MS, DD, DP], f32r, tag="wt_s")
                ut_mm = big.tile([128, NCHUNK, DP, B], f32r, tag="ut_mm")
                nc.vector.tensor_copy(ut_mm[:], ut[:])  # rounds to f32r
            else:
                wt_s = wt
                ut_mm = ut

            ps = psum.tile([128, DP, MS, DD], f32, tag="ps")
            for c in range(NCHUNK):
                # scale W by (1 + B_prior), per (n, chunk, m-slot); for f32r
                # this op also performs the required rounding on its output
                for ms in range(MS):
                    nc.vector.tensor_scalar_mul(
                        wt_s[:, c, ms], wt[:, c, ms], cb1[:, c, ms : ms + 1]
                    )
                nc.tensor.matmul(
                    ps[:],
                    ut_mm[:, c].rearrange("n p b -> n (p b)"),
                    wt_s[:, c].rearrange("n m d p -> n p m d"),
                    start=(c == 0),
                    stop=(c == NCHUNK - 1),
                )

            # diagonal extraction: S[b, m', d] = sum_p ps[16p+b, p, :, :].
            # DVE/walrus reject partition bases that aren't 32-aligned, so the
            # cross-partition gather runs on the PE: out2[b,:] accumulates
            # SEL[:, p].T @ ps_sb[:, p] over p, where SEL[q,p,b] = (q==16p+b).
            f32t = f32
            ps_sb = small.tile([128, DP, MS, DD], f32, tag="ps_sb")
            nc.vector.tensor_copy(ps_sb[:], ps[:])
            ps2 = psum.tile([B, MS, DD], f32, tag="ps2")
            for p in range(DP):
                nc.tensor.matmul(
                    ps2[:],
                    sel[:, p],
                    ps_sb[:, p],
                    start=(p == 0),
                    stop=(p == DP - 1),
                )
            s = small.tile([B, MS, DD], f32t, tag="s")
            nc.vector.tensor_copy(s[:], ps2[:])

            # squash over d per (b, m-slot)
            sq = small.tile([B, MS, DD], f32t, tag="sq")
            nc.vector.tensor_mul(sq[:], s[:], s[:])
            n2 = small.tile([B, MS], f32t, tag="n2")
            nc.vector.tensor_reduce(
                n2[:], sq[:], axis=mybir.AxisListType.X, op=mybir.AluOpType.add
            )
            nrm = small.tile([B, MS], f32t, tag="nrm")
            nc.scalar.sqrt(nrm[:], n2[:])
            e = small.tile([B, MS], f32t, tag="e")
            nc.scalar.activation(
                e[:], nrm[:], mybir.ActivationFunctionType.Exp, scale=-1.0
            )
            coef = small.tile([B, MS], f32t, tag="coef")
            nc.vector.tensor_scalar(
                coef[:], e[:], -1.0, 1.0, mybir.AluOpType.mult, mybir.AluOpType.add
            )
            neps = small.tile([B, MS], f32t, tag="neps")
            nc.vector.tensor_scalar_add(neps[:], nrm[:], EPS)
            rec = small.tile([B, MS], f32t, tag="rec")
            nc.vector.reciprocal(rec[:], neps[:])
            fac = small.tile([B, MS], f32t, tag="fac")
            nc.vector.tensor_mul(fac[:], coef[:], rec[:])
            o = small.tile([B, MS, DD], f32, tag="o")
            for ms in range(MS):
                nc.vector.tensor_scalar_mul(o[:, ms], s[:, ms], fac[:, ms : ms + 1])
            nc.sync.dma_start(out_ap[:], o[:])

    nc.compile()
    return nc


def make_in_maps(primary_caps: np.ndarray, W: np.ndarray, B_prior: np.ndarray):
    GROUPS = [(0, 1), (1, 5), (5, 9)]
    u = np.asarray(primary_caps, dtype=np.float32)
    # u_h [n', c, p, b]
    u_h = np.ascontiguousarray(
        u.transpose(1, 2, 0).reshape(NCHUNK, 128, DP, B).transpose(1, 0, 2, 3)
    )
    sel = np.zeros((128, DP, B), dtype=np.float32)
    for p in range(DP):
        for b in range(B):
            sel[16 * p + b, p, b] = 1.0
    Wf = np.asarray(W, dtype=np.float32)
    Bf = np.asarray(B_prior, dtype=np.float32)
    in_maps = []
    for pr in M_PAIRS:
        wp = Wf[list(pr)]  # [MS, N, DD, DP]
        # W_h [ms, flat group-major (n', c_in_g, d*p)]
        parts = []
        for ms in range(MS):
            row = []
            for c0, c1 in GROUPS:
                blk = wp[ms, c0 * 128 : c1 * 128].reshape(c1 - c0, 128, DD * DP)
                row.append(blk.transpose(1, 0, 2).reshape(-1))
            parts.append(np.concatenate(row))
        w_h = np.ascontiguousarray(np.stack(parts))
        bp = Bf[list(pr), 0, :]  # [MS, N]
        bp_h = np.ascontiguousarray(
            bp.T.reshape(NCHUNK, 128, MS).transpose(1, 0, 2)
        )
        in_maps.append({"W_h": w_h, "u_h": u_h, "bp_h": bp_h, "SEL": sel})
    return in_maps


def kernel(primary_caps: np.ndarray, W: np.ndarray, B_prior: np.ndarray) -> np.ndarray:
    from concourse.bass_utils import run_bass_kernel_spmd

    global _compiled
    if _compiled is None:
        _compiled = _build_raw()
    nc = _compiled

    in_maps = make_in_maps(primary_caps, W, B_prior)
    res = run_bass_kernel_spmd(nc, in_maps, list(range(8))).results
    out = np.empty((B, M, DD), dtype=np.float32)
    for i in range(5):
        out[:, 2 * i : 2 * i + 2, :] = res[i]["out_s"]
    return out
